# revision 1
# baseline (speedup 1.0000x reference)
"""Trainium2 Bass kernel for nn_Algebraic_interval: t-norm feature expansion.

For each input x in {xl, xu} of shape [65536, 16], computes
  out = concat([x, prod(x[:, idx2], -1), prod(x[:, idx3], -1)], axis=1)
over all C(16,2)=120 pair and C(16,3)=560 triple column combinations,
giving two [65536, 696] outputs.

Strategy (pure data parallel over 8 cores, 8192 rows each):
  - Products are computed as exp(G^T @ ln(x)): TensorE gathers/sums the
    logs through a static 0/1 combination matrix G, ScalarE does ln/exp.
  - fp32 matmuls run at 1/4 rate on the PE (two-pass decomposition), so
    ln(x) is split into three bf16 components h1+h2+h3 (~24 mantissa
    bits) stacked along the contraction dim: K=96 = 3 passes x 32
    features (16 xl + 16 xu interleaved; zero rows in G select the
    tensor). One full-rate bf16 matmul per 512-column chunk then
    reconstructs G @ ln(x) in fp32 PSUM (G entries are exact in bf16,
    so products are exact; only the 3-way bf16 split truncates).
  - The zero-clamp is folded into the Ln activation's bias:
    ln(x + 1e-30) is exact for every representable nonzero uniform
    value and gives a finite -69.1 for x=0, so exp underflows to the
    exact 0 product.
  - One flat [128, 1392] exp per 128-row tile writes both outputs' rows
    into an SBUF slab; slabs (1 tile at the stream edges for fast
    ramp/drain, 2 tiles in steady state) stream out as single DMAs into
    an interleaved out[row, {l,u}, 696] DRAM tensor, giving 5568-byte
    contiguous descriptors on both sides (l/u are split on the host
    afterwards). The h1 cast runs on ScalarE right after Ln to avoid a
    cross-engine sem hop on the startup critical path.

Host-side: inputs are pre-transposed to feature-major xt[32, 8192]
(partition p<16: xl feature p; p>=16: xu feature p-16) per core.
"""

import itertools
import numpy as np

N_COLS = 16
B_FULL = 65536
N_CORES = 8
B_CORE = B_FULL // N_CORES          # 8192
PAIRS = list(itertools.combinations(range(N_COLS), 2))    # 120
TRIPLES = list(itertools.combinations(range(N_COLS), 3))  # 560
N_OUT = N_COLS + len(PAIRS) + len(TRIPLES)                # 696
TILES_PER_CORE = B_CORE // 128      # 64
TILES_PER_SLAB = 2
N_SLABS = TILES_PER_CORE // TILES_PER_SLAB  # 32
# matmul output chunking over the 2*696 concatenated columns (PSUM banks)
CHUNKS = [(0, 512), (512, 512), (1024, 368)]
# prologue batch-dim chunking (columns of xt); first chunks small so the
# matmul pipeline starts early
CHUNK_COLS = [1024, 1024, 3072, 3072]

_CACHED = {}


def _make_g() -> np.ndarray:
    """[96, 2*696] bf16 0/1 matrix, 3 vertical copies of [32, 1392].

    Rows (within a 32-block): 0..15 select xl features, 16..31 xu.
    Columns 0:696 are out_l (16 singles | 120 pairs | 560 triples, lex
    order), columns 696:1392 are out_u.
    """
    import ml_dtypes

    g = np.zeros((32, 2 * N_OUT), dtype=np.float32)
    for half, row0 in ((0, 0), (1, 16)):
        c0 = half * N_OUT
        for j in range(N_COLS):
            g[row0 + j, c0 + j] = 1.0
        for idx, pair in enumerate(PAIRS):
            for f in pair:
                g[row0 + f, c0 + N_COLS + idx] = 1.0
        for idx, tri in enumerate(TRIPLES):
            for f in tri:
                g[row0 + f, c0 + N_COLS + len(PAIRS) + idx] = 1.0
    return np.tile(g, (3, 1)).astype(ml_dtypes.bfloat16)


def _build_program():
    import concourse.bacc as bacc
    import concourse.mybir as mybir
    import concourse.tile as tile
    from concourse.bass import MemorySpace

    f32 = mybir.dt.float32
    bf16 = mybir.dt.bfloat16
    Act = mybir.ActivationFunctionType
    nc = bacc.Bacc("TRN2", target_bir_lowering=False, debug=False)

    # const AP for the Ln bias (1e-30 is normal fp32, so no FTZ risk;
    # ln(0 + 1e-30) = -69.08 and exp of any sum including it underflows
    # to the exact 0 product)
    _c = nc.alloc_sbuf_tensor("const-float32-tiny", [128, 1], f32)
    nc.gpsimd.memset(_c.ap(), 1e-30)
    nc.const_aps.aps[(f32, 1e-30)] = _c.ap()

    xt = nc.dram_tensor("xt", [32, B_CORE], f32, kind="ExternalInput")
    out_lu = nc.dram_tensor(
        "out_lu", [B_CORE, 2, N_OUT], f32, kind="ExternalOutput"
    )
    gm = nc.inline_tensor(_make_g(), name="gmat")

    with tile.TileContext(nc) as tc:
        with (
            tc.tile_pool(name="const", bufs=1) as const_pool,
            tc.tile_pool(name="inp", bufs=1) as inp_pool,
            tc.tile_pool(name="scratch", bufs=2) as scratch_pool,
            tc.tile_pool(name="slab", bufs=6) as slab_pool,
            tc.tile_pool(name="psum", bufs=2, space=MemorySpace.PSUM) as psum_pool,
        ):
            gm_sb = const_pool.tile([96, 2 * N_OUT], bf16)

            # ln(x + 1e-30), then 3-way bf16 split of the logs:
            # h1=bf16(ln), h2=bf16(ln-h1), h3=bf16(ln-h1-h2).
            # DVE ops cannot cross partitions, so h2/h3 are computed on
            # partitions 0:32 and DMA'd into the stacked weight buffer.
            # Chunked along the batch dim so matmuls can start early.
            w_chunks = []   # (tile, n_128tiles)
            col0 = 0
            for j, cw_cols in enumerate(CHUNK_COLS):
                cols = slice(col0, col0 + cw_cols)
                col0 += cw_cols
                xt_sb = scratch_pool.tile([32, cw_cols], f32, tag="xt_sb")
                nc.sync.dma_start(xt_sb[:], xt[:, cols])
                if j == 0:
                    nc.sync.dma_start(gm_sb[:], gm[:])
                lnx = scratch_pool.tile([32, cw_cols], f32, tag="lnx")
                nc.scalar.activation(lnx[:], xt_sb[:], Act.Ln, bias=1e-30)
                w = inp_pool.tile([96, cw_cols], bf16, tag=f"w{j}")
                # h1 cast on ScalarE: back-to-back with Ln, no sem hop
                nc.scalar.copy(w[0:32, :], lnx[:])
                r1 = scratch_pool.tile([32, cw_cols], f32, tag="r1")
                nc.vector.tensor_sub(r1[:], lnx[:], w[0:32, :])
                h2 = scratch_pool.tile([32, cw_cols], bf16, tag="h2")
                nc.vector.tensor_copy(h2[:], r1[:])
                h3 = scratch_pool.tile([32, cw_cols], bf16, tag="h3")
                nc.vector.tensor_sub(h3[:], r1[:], h2[:])
                nc.sync.dma_start(w[32:64, :], h2[:])
                nc.sync.dma_start(w[64:96, :], h3[:])
                w_chunks.append((w, cw_cols // 128))

            # tile index -> (chunk tile, local column offset)
            tile_map = []
            for w, ntiles in w_chunks:
                for i in range(ntiles):
                    tile_map.append((w, i * 128))

            ovp = out_lu.ap().rearrange("(t p) two c -> p t two c", p=128)

            slab_sizes = [1, 1] + [2] * 30 + [1, 1]
            t0 = 0
            for sz in slab_sizes:
                slab = slab_pool.tile([128, sz, 2 * N_OUT], f32, tag="ot")
                for q in range(sz):
                    w, o = tile_map[t0 + q]
                    lhsT = w[:, o : o + 128]
                    S = psum_pool.tile([128, 1536], f32)
                    for c0, cw in CHUNKS:
                        nc.tensor.matmul(
                            S[:, c0 : c0 + cw],
                            lhsT,
                            gm_sb[:, c0 : c0 + cw],
                        )
                    nc.scalar.activation(
                        slab[:, q, :], S[:, 0 : 2 * N_OUT], Act.Exp
                    )
                nc.sync.dma_start(
                    ovp[:, t0 : t0 + sz],
                    slab[:].rearrange("p q (two c) -> p q two c", two=2),
                )
                t0 += sz

    nc.compile()
    return nc


def _spot_check(xl, xu, full_l, full_u, n_rows=48) -> bool:
    """Validate sampled rows against an exact host-side recomputation."""
    if not (np.isfinite(full_l).all() and np.isfinite(full_u).all()):
        return False
    rows = np.linspace(0, B_FULL - 1, n_rows, dtype=np.int64)
    idx2 = np.array(PAIRS)
    idx3 = np.array(TRIPLES)
    for x, out in ((xl, full_l), (xu, full_u)):
        xs = x[rows].astype(np.float64)
        exp = np.concatenate(
            [xs, np.prod(xs[:, idx2], -1), np.prod(xs[:, idx3], -1)], axis=1
        )
        rel = np.abs(out[rows] - exp) / np.maximum(np.abs(exp), 1e-9)
        if rel.max() > 1e-3:
            return False
    return True


def kernel(xl, xu):
    from concourse.bass_utils import run_bass_kernel_spmd

    xl = np.asarray(xl, dtype=np.float32)
    xu = np.asarray(xu, dtype=np.float32)

    if "nc" not in _CACHED:
        _CACHED["nc"] = _build_program()
    nc = _CACHED["nc"]

    in_maps = []
    for i in range(N_CORES):
        lo, hi = i * B_CORE, (i + 1) * B_CORE
        xt = np.concatenate([xl[lo:hi].T, xu[lo:hi].T], axis=0)
        in_maps.append({"xt": np.ascontiguousarray(xt)})

    # retry loop: guards against rare transient device/DMA corruption
    last_err = None
    full_l = full_u = None
    for attempt in range(3):
        try:
            res = run_bass_kernel_spmd(nc, in_maps, list(range(N_CORES)))
        except Exception as e:  # transient device error: retry
            last_err = e
            import time

            time.sleep(3)
            continue
        full = np.concatenate(
            [res.results[i]["out_lu"] for i in range(N_CORES)], axis=0
        )
        full_l = np.ascontiguousarray(full[:, 0, :])
        full_u = np.ascontiguousarray(full[:, 1, :])
        if _spot_check(xl, xu, full_l, full_u):
            return full_l, full_u
    if full_l is None:
        raise last_err
    return full_l, full_u

